# revision 1
# baseline (speedup 1.0000x reference)
"""CrossAttentionBlock Trainium2 kernel, 8-core SPMD.

Sharding: (batch=4) x (seq halves=2) -> 8 cores, each core computes one
batch's half of the S=2048 query rows end-to-end (QKV proj, cross-attn,
output proj, residual + layernorm). No collectives.

All matmuls in bf16 (full PE rate, halved DMA vs f32). Attention computes
scores TRANSPOSED (k stationary, q moving -> scoresT[t,s]) so probs feed
the ctx matmul directly as the moving operand: no PE transposes, no
PSUM->SBUF probs copies. Softmax denominators come out partition-broadcast
via an all-ones stationary matmul, so normalization is one DVE mult fused
into the ctx PSUM->SBUF copy.
"""
import numpy as np

B, S, T, H, NH = 4, 2048, 256, 2048, 16
HD = H // NH  # 128
P = 128
MH = S // 2  # rows per core = 1024
LN_EPS = 1e-5
ISQ = 1.0 / np.sqrt(HD)

_CACHE = {}


def _build2(reps=1, with_mask=True, affine=True):
    from contextlib import ExitStack
    import concourse.bass as bass
    from concourse import bacc
    import concourse.mybir as mybir
    import concourse.tile as tile

    f32 = mybir.dt.float32
    bf16 = mybir.dt.bfloat16
    Alu = mybir.AluOpType
    Act = mybir.ActivationFunctionType

    nc = bacc.Bacc("TRN2", target_bir_lowering=False, debug=False, num_devices=8)
    KO = H // P  # 16
    XTp = nc.declare_dram_parameter("XT", [H, MH], bf16, isOutput=False)
    Xresp = nc.declare_dram_parameter("Xres", [MH, H], bf16, isOutput=False)
    ATp = nc.declare_dram_parameter("AT", [H, T], bf16, isOutput=False)
    Wp = {w: nc.declare_dram_parameter(w, [H, H], bf16, isOutput=False)
          for w in ("WvT", "WoT")}
    Wp["WqT"] = nc.declare_dram_parameter("WqT", [KO, P, KO, P], bf16, isOutput=False)
    Wp["WkT"] = nc.declare_dram_parameter("WkT", [KO, P, KO, P], bf16, isOutput=False)
    bqp = nc.declare_dram_parameter("bq", [H], f32, isOutput=False)
    bkp = nc.declare_dram_parameter("bk", [H], f32, isOutput=False)
    bvp = nc.declare_dram_parameter("bv", [H], f32, isOutput=False)
    mrowp = nc.declare_dram_parameter("mrow", [1, T], f32, isOutput=False)
    lngp = nc.declare_dram_parameter("ln_g", [H], f32, isOutput=False)
    lnbp = nc.declare_dram_parameter("ln_b", [H], f32, isOutput=False)
    OUTp = nc.declare_dram_parameter("OUT", [MH, H], f32, isOutput=True)

    ctx = ExitStack()
    with tile.TileContext(nc) as tc, ctx:
        if reps > 1:
            ctx.enter_context(tc.For_i(0, reps, 1))
        persist = ctx.enter_context(tc.tile_pool(name="persist", bufs=1))
        wstream = ctx.enter_context(tc.tile_pool(name="wstream", bufs=4))
        small = ctx.enter_context(tc.tile_pool(name="small", bufs=1))
        attnsb = ctx.enter_context(tc.tile_pool(name="attnsb", bufs=3))
        sums_p = ctx.enter_context(tc.tile_pool(name="sums", bufs=4))
        bcast = ctx.enter_context(tc.tile_pool(name="bcast", bufs=2))
        psA = ctx.enter_context(tc.tile_pool(name="psA", bufs=4, space="PSUM"))
        psB = ctx.enter_context(tc.tile_pool(name="psB", bufs=4, space="PSUM"))

        # --- constants ---
        eps_t = small.tile([P, 1], f32, tag="eps")
        nc.vector.memset(eps_t[:], LN_EPS)
        ones_m = small.tile([P, P], bf16, tag="ones_m")
        nc.vector.memset(ones_m[:], 1.0)
        if with_mask:
            # mask per key position t, laid out [t%128, t//128]
            mrow_t = small.tile([P, 2], f32, tag="mrowT")
            nc.sync.dma_start(mrow_t[:], mrowp.ap().rearrange("1 (c p) -> p c", p=P))
        if affine:
            bq_t = small.tile([P, KO], f32, tag="bq")
            bk_t = small.tile([P, KO], f32, tag="bk")
            nc.sync.dma_start(bq_t[:], bqp.ap().rearrange("(o p) -> p o", p=P))
            nc.sync.dma_start(bk_t[:], bkp.ap().rearrange("(o p) -> p o", p=P))

        def bc_tile(src_ap):
            t = bcast.tile([P, H], f32, tag="bc", name="bc")
            bcast_ap = bass.AP(tensor=src_ap.tensor, offset=src_ap.offset,
                               ap=[[0, P]] + src_ap.ap)
            nc.sync.dma_start(t[:], bcast_ap)
            return t

        bvbc = bc_tile(bvp.ap()) if affine else None

        # --- persistent tensors ---
        kT = persist.tile([P, KO, T], bf16, tag="kT")
        v_t = [persist.tile([P, H], bf16, tag=f"v{t}", name=f"v{t}") for t in range(2)]
        qca = [persist.tile([P, 512], bf16, tag=f"qca{i}", name=f"qca{i}") for i in range(KO)]
        qcb = [persist.tile([P, 512], bf16, tag=f"qcb{i}", name=f"qcb{i}") for i in range(KO)]
        qhalf = [qca, qcb]
        at_t = persist.tile([P, KO, T], bf16, tag="at", name="at_t")
        for q4 in range(4):
            nc.scalar.dma_start(at_t[:, q4 * 4:(q4 + 1) * 4],
                                ATp.ap()[q4 * 512:(q4 + 1) * 512]
                                .rearrange("(o p) t -> p o t", p=P))

        # ---------------- emit helpers ----------------
        def emit_k(hp):
            wk = wstream.tile([P, KO, P], bf16, tag="w", name="wk")
            weng = nc.sync if hp % 2 == 0 else nc.gpsimd
            weng.dma_start(wk[:], Wp["WkT"].ap()[hp])
            pool = psA if hp % 2 == 0 else psB
            kps = pool.tile([P, 512], f32, tag="ps", name="kps")
            for h in range(KO):
                nc.tensor.matmul(kps[:, :T], wk[:, h], at_t[:, h],
                                 start=(h == 0), stop=(h == KO - 1))
            if affine:
                nc.vector.tensor_tensor(out=kT[:, hp], in0=kps[:, :T],
                                        in1=bk_t[:, hp:hp + 1].to_broadcast((P, T)),
                                        op=Alu.add)
            else:
                nc.vector.tensor_copy(kT[:, hp], kps[:, :T])

        def emit_v(n):
            pool = psA if n % 2 == 0 else psB
            vps = [pool.tile([P, 512], f32, tag="ps", name="vps") for _ in range(2)]
            for hg in range(4):
                wv = wstream.tile([P, 4, 512], bf16, tag="w", name="wv")
                weng = nc.gpsimd if hg % 2 == 0 else nc.scalar
                weng.dma_start(wv[:], Wp["WvT"].ap()[hg * 512:(hg + 1) * 512,
                                                          n * 512:(n + 1) * 512]
                                      .rearrange("(g p) c -> p g c", p=P))
                for t in range(2):
                    for g in range(4):
                        h = hg * 4 + g
                        nc.tensor.matmul(vps[t][:], at_t[:, h, t * P:(t + 1) * P],
                                         wv[:, g], start=(h == 0), stop=(h == KO - 1))
            for t in range(2):
                if affine:
                    nc.vector.tensor_tensor(out=v_t[t][:, n * 512:(n + 1) * 512],
                                            in0=vps[t][:],
                                            in1=bvbc[:, n * 512:(n + 1) * 512], op=Alu.add)
                else:
                    nc.vector.tensor_copy(v_t[t][:, n * 512:(n + 1) * 512], vps[t][:])

        def emit_q_both(hp):
            wq = wstream.tile([P, KO, P], bf16, tag="w", name="wq")
            weng = nc.sync if hp % 2 == 0 else nc.scalar
            weng.dma_start(wq[:], Wp["WqT"].ap()[hp])
            qp = [psA.tile([P, 512], f32, tag="ps", name="qps"),
                  psB.tile([P, 512], f32, tag="ps", name="qps")]
            for m in range(2):
                for h in range(KO):
                    nc.tensor.matmul(qp[m][:], wq[:, h], xt_t[:, h, m * 512:(m + 1) * 512],
                                     start=(h == 0), stop=(h == KO - 1))
            for m in range(2):
                if affine:
                    nc.vector.tensor_tensor(out=qhalf[m][hp][:], in0=qp[m][:],
                                            in1=bq_t[:, hp:hp + 1].to_broadcast((P, 512)),
                                            op=Alu.add)
                else:
                    nc.vector.tensor_copy(qhalf[m][hp][:], qp[m][:])

        # attention unit (head hd, query half m): scoresT -> exp -> sums -> ctx
        def attn_A(hd, m):
            sp = [psA.tile([P, 512], f32, tag="ps", name="sps") for _ in range(2)]
            expT = attnsb.tile([P, 2, 512], bf16, tag="expT", name="expT")
            qh = qhalf[m][hd]
            for c in range(2):
                nc.tensor.matmul(sp[c][:], kT[:, hd, c * P:(c + 1) * P], qh[:],
                                 start=True, stop=True)
                if with_mask:
                    nc.scalar.activation(expT[:, c], sp[c][:], Act.Exp,
                                         scale=float(ISQ), bias=mrow_t[:, c:c + 1])
                else:
                    nc.scalar.activation(expT[:, c], sp[c][:], Act.Exp,
                                         scale=float(ISQ))
            return (hd, m, expT)

        def attn_B(state):
            hd, m, expT = state
            sm = psA.tile([P, 512], f32, tag="ps", name="smps")
            for c in range(2):
                nc.tensor.matmul(sm[:], ones_m[:], expT[:, c],
                                 start=(c == 0), stop=(c == 1))
            cx = psB.tile([P, 512], f32, tag="ps", name="cxps")
            for c in range(2):
                nc.tensor.matmul(cx[:], v_t[c][:, hd * P:(hd + 1) * P], expT[:, c],
                                 start=(c == 0), stop=(c == 1))
            rb = attnsb.tile([P, 512], f32, tag="rb", name="rb")
            nc.vector.reciprocal(rb[:], sm[:])
            nc.vector.tensor_tensor(out=qhalf[m][hd][:], in0=cx[:], in1=rb[:],
                                    op=Alu.mult)

        def emit_o(mg, n):
            pool = psA if n % 2 == 0 else psB
            ops = [pool.tile([P, 512], f32, tag="ps", name="ops") for _ in range(4)]
            for hg in range(4):
                wo = wstream.tile([P, 4, 512], bf16, tag="w", name="wo")
                weng = nc.gpsimd if hg % 2 == 0 else nc.sync
                weng.dma_start(wo[:], Wp["WoT"].ap()[hg * 512:(hg + 1) * 512,
                                                          n * 512:(n + 1) * 512]
                                    .rearrange("(g p) c -> p g c", p=P))
                for mi in range(4):
                    m = mg * 4 + mi
                    for g in range(4):
                        hp = hg * 4 + g
                        nc.tensor.matmul(ops[mi][:],
                                         qhalf[m // 4][hp][:, (m % 4) * P:(m % 4 + 1) * P],
                                         wo[:, g], start=(hp == 0), stop=(hp == KO - 1))
            xr = wstream.tile([P, 4, 512], bf16, tag="w", name="xr")
            nc.scalar.dma_start(xr[:], Xresp.ap()[mg * 512:(mg + 1) * 512,
                                                  n * 512:(n + 1) * 512]
                                .rearrange("(g p) c -> p g c", p=P))
            for mi in range(4):
                m = mg * 4 + mi
                nc.vector.tensor_tensor(out=out_t[:, m, n * 512:(n + 1) * 512],
                                        in0=ops[mi][:], in1=xr[:, mi], op=Alu.add)

        def emit_ln(m):
            row = out_t[:, m]
            stats = sums_p.tile([P, 4, 6], f32, tag="bnst", name="stats")
            for q in range(4):
                nc.vector.bn_stats(out=stats[:, q], in_=row[:, q * 512:(q + 1) * 512])
            mv = sums_p.tile([P, 2], f32, tag="bnmv", name="mv")
            nc.vector.bn_aggr(out=mv[:], in_=stats[:])
            std = sums_p.tile([P, 1], f32, tag="std", name="std")
            nc.scalar.activation(std[:], mv[:, 1:2], Act.Sqrt, bias=eps_t[:])
            rstd = sums_p.tile([P, 1], f32, tag="rstd", name="rstd")
            nc.vector.reciprocal(rstd[:], std[:])
            nc.vector.tensor_scalar(out=row, in0=row, scalar1=mv[:, 0:1],
                                    scalar2=rstd[:], op0=Alu.subtract, op1=Alu.mult)
            if affine:
                nc.vector.tensor_tensor(out=row, in0=row, in1=gbc[:], op=Alu.mult)
                nc.vector.tensor_tensor(out=row, in0=row, in1=bbc[:], op=Alu.add)
            oeng = nc.scalar if m >= 4 else nc.sync
            oeng.dma_start(OUTp.ap()[m * P:(m + 1) * P, :], row)

        # ---------------- schedule ----------------
        for hp in range(KO):
            emit_k(hp)

        xt_t = persist.tile([P, KO, MH], bf16, tag="xt", name="xt_t")
        for q4 in range(4):
            nc.scalar.dma_start(
                xt_t[:, q4 * 4:(q4 + 1) * 4],
                XTp.ap()[q4 * 512:(q4 + 1) * 512]
                .rearrange("(o p) m -> p o m", p=P))

        out_t = persist.tile([P, 8, H], f32, tag="out", name="out_t")
        gbc = bc_tile(lngp.ap()) if affine else None
        bbc = bc_tile(lnbp.ap()) if affine else None

        # V-proj + Q-proj + attention all pipelined: V rides the first 4
        # iterations; attn for hp-1 runs under Q for hp. attn_B(hd) only
        # needs v chunk hd//4, emitted >= 1 iteration earlier.
        pend = None
        prev = None
        for hp in range(KO):
            emit_q_both(hp)
            if hp < 4:
                emit_v(hp)
            if prev is not None:
                for m in range(2):
                    st = attn_A(prev, m)
                    if pend is not None:
                        attn_B(pend)
                    pend = st
            prev = hp
        for m in range(2):
            st = attn_A(prev, m)
            if pend is not None:
                attn_B(pend)
            pend = st
        attn_B(pend)

        # O-proj + LN; rows 0-3 LN chase the mg=1 matmuls
        for n in range(4):
            emit_o(0, n)
        emit_o(1, 0)
        emit_ln(0)
        emit_ln(3)
        emit_o(1, 1)
        emit_ln(1)
        emit_o(1, 2)
        emit_ln(2)
        # last o-group: all 4 WoT tiles prefetched, pure m-major so each
        # out row retires every 16 matmuls and LN chases the PE
        wos = []
        for hg in range(4):
            wo = wstream.tile([P, 4, 512], bf16, tag="w", name="wo")
            weng = nc.gpsimd if hg % 2 == 0 else nc.sync
            weng.dma_start(wo[:], Wp["WoT"].ap()[hg * 512:(hg + 1) * 512,
                                                 1536:2048]
                           .rearrange("(g p) c -> p g c", p=P))
            wos.append(wo)
        xr = wstream.tile([P, 4, 512], bf16, tag="w", name="xr")
        nc.scalar.dma_start(xr[:], Xresp.ap()[512:1024, 1536:2048]
                            .rearrange("(g p) c -> p g c", p=P))
        ops = [psB.tile([P, 512], f32, tag="ps", name="ops") for _ in range(4)]
        for mi in range(4):
            m = 4 + mi
            for hp in range(KO):
                nc.tensor.matmul(ops[mi][:],
                                 qhalf[1][hp][:, mi * P:(mi + 1) * P],
                                 wos[hp // 4][:, hp % 4],
                                 start=(hp == 0), stop=(hp == KO - 1))
            nc.vector.tensor_tensor(out=out_t[:, m, 1536:2048],
                                    in0=ops[mi][:], in1=xr[:, mi], op=Alu.add)
            emit_ln(m)

    nc.finalize()
    return nc


def _get_nc(reps=1, with_mask=False, affine=True):
    key = f"nc{reps}_{with_mask}_{affine}"
    if key not in _CACHE:
        _CACHE[key] = _build2(reps, with_mask, affine)
    return _CACHE[key]


_SHARDED = {"XT", "Xres", "AT", "mrow"}


def _get_runner(reps=1, with_mask=False, affine=True):
    key = f"runner{reps}_{with_mask}_{affine}"
    if key in _CACHE:
        return _CACHE[key]
    import jax
    from jax.sharding import Mesh, PartitionSpec, NamedSharding
    try:
        from jax.experimental.shard_map import shard_map
    except ImportError:
        from jax import shard_map
    from concourse.bass2jax import (_bass_exec_p, partition_id_tensor,
                                    install_neuronx_cc_hook)
    import concourse.mybir as mybir

    install_neuronx_cc_hook()
    nc = _get_nc(reps, with_mask, affine)
    partition_name = nc.partition_id_tensor.name if nc.partition_id_tensor else None
    in_names, out_names, out_avals = [], [], []
    for alloc in nc.m.functions[0].allocations:
        if not isinstance(alloc, mybir.MemoryLocationSet):
            continue
        name = alloc.memorylocations[0].name
        if alloc.kind == "ExternalInput":
            if name != partition_name:
                in_names.append(name)
        elif alloc.kind == "ExternalOutput":
            out_names.append(name)
            out_avals.append(jax.core.ShapedArray(tuple(alloc.tensor_shape),
                                                  mybir.dt.np(alloc.dtype)))

    bind_in_names = list(in_names) + ([partition_name] if partition_name else [])

    def _body(*args):
        operands = list(args)
        if partition_name is not None:
            operands.append(partition_id_tensor())
        outs = _bass_exec_p.bind(
            *operands, out_avals=tuple(out_avals),
            in_names=tuple(bind_in_names), out_names=tuple(out_names),
            lowering_input_output_aliases=(),
            sim_require_finite=True, sim_require_nnan=True, nc=nc)
        return tuple(outs)

    devices = jax.devices()[:8]
    mesh = Mesh(np.asarray(devices), ("core",))
    in_specs = tuple(PartitionSpec("core") if n in _SHARDED else PartitionSpec()
                     for n in in_names)
    out_specs = tuple(PartitionSpec("core") for _ in out_names)
    fn = jax.jit(shard_map(_body, mesh=mesh, in_specs=in_specs,
                           out_specs=out_specs, check_rep=False),
                 keep_unused=True)
    shardings = {n: NamedSharding(mesh, s) for n, s in zip(in_names, in_specs)}
    _CACHE[key] = (fn, in_names, mesh, shardings)
    return _CACHE[key]


def _host_args(hidden_states, audio_tokens, attention_mask, Wq, bq, Wk, bk, Wv,
               bv, Wo, bo, ln_g, ln_b):
    import ml_dtypes
    bf = ml_dtypes.bfloat16
    hs = np.asarray(hidden_states, np.float32)
    at = np.asarray(audio_tokens, np.float32)
    am = np.asarray(attention_mask, np.float32)
    Wq = np.asarray(Wq, np.float32); Wk = np.asarray(Wk, np.float32)
    Wv = np.asarray(Wv, np.float32); Wo = np.asarray(Wo, np.float32)
    bq = np.asarray(bq, np.float32); bk = np.asarray(bk, np.float32)
    bv = np.asarray(bv, np.float32); bo = np.asarray(bo, np.float32)
    ln_g = np.asarray(ln_g, np.float32); ln_b = np.asarray(ln_b, np.float32)

    def _tile_w(WT):
        # [h, h'] -> [hp, p, o, c] with h = o*128+p, h' = hp*128+c
        return np.ascontiguousarray(
            WT.reshape(KO_, P, KO_, P).transpose(2, 1, 0, 3)).astype(bf)

    KO_ = H // P
    vals = {
        "WqT": _tile_w(Wq.T), "WkT": _tile_w(Wk.T),
        "WvT": np.ascontiguousarray(Wv.T).astype(bf),
        "WoT": np.ascontiguousarray(Wo.T).astype(bf),
        "bq": bq, "bk": bk, "bv": bv, "ln_g": ln_g, "ln_b": ln_b,
    }
    xts, xrs, ats, mrs = [], [], [], []
    for c in range(8):
        b, half = divmod(c, 2)
        xs = hs[b, half * MH:(half + 1) * MH]
        xts.append(xs.T.astype(bf))
        xrs.append((xs + bo).astype(bf))
        ats.append(at[b].T.astype(bf))
        mrs.append((am[b] * -1e9).reshape(1, T))
    vals["XT"] = np.concatenate(xts, axis=0)
    vals["Xres"] = np.concatenate(xrs, axis=0)
    vals["AT"] = np.concatenate(ats, axis=0)
    vals["mrow"] = np.concatenate(mrs, axis=0)
    return vals


def _assemble(out_global):
    o = np.asarray(out_global).reshape(8, MH, H)
    out = np.empty((B, S, H), np.float32)
    for c in range(8):
        b, half = divmod(c, 2)
        out[b, half * MH:(half + 1) * MH] = o[c]
    return out


def _flags(inputs):
    with_mask = bool(np.any(np.asarray(inputs["attention_mask"]) != 0))
    affine = not (np.all(np.asarray(inputs["bq"]) == 0)
                  and np.all(np.asarray(inputs["bk"]) == 0)
                  and np.all(np.asarray(inputs["bv"]) == 0)
                  and np.all(np.asarray(inputs["ln_g"]) == 1)
                  and np.all(np.asarray(inputs["ln_b"]) == 0))
    return with_mask, affine


def kernel(**inputs):
    with_mask, affine = _flags(inputs)
    fn, in_names, mesh, shardings = _get_runner(1, with_mask, affine)
    vals = _host_args(**inputs)
    outs = fn(*[vals[n] for n in in_names])
    return _assemble(outs[0])


def device_args(inputs, reps=1):
    """device_put all inputs once; returns list for run_device."""
    import jax
    with_mask, affine = _flags(inputs)
    fn, in_names, mesh, shardings = _get_runner(reps, with_mask, affine)
    vals = _host_args(**inputs)
    return [jax.device_put(vals[n], shardings[n]) for n in in_names]


def run_device(args, reps=1, with_mask=False, affine=False):
    import jax
    fn, in_names, mesh, shardings = _get_runner(reps, with_mask, affine)
    outs = fn(*args)
    jax.block_until_ready(outs)
    return outs



# revision 2
# speedup vs baseline: 1.1326x; 1.1326x over previous
"""CrossAttentionBlock Trainium2 kernel, 8-core SPMD, fp8-DoubleRow value path.

Sharding: (batch=4) x (seq halves=2) -> 8 cores, each core computes one
batch's half of the S=2048 query rows end-to-end (QKV proj, cross-attn,
output proj, residual + layernorm). No collectives.

Precision: the score path (Q/K projections, Q.K^T scores) runs in bf16 --
softmax amplifies score errors so it needs the bits. The value path
(V projection, softmax-weighted sum, output projection) runs in fp8 e4m3
with DoubleRow matmuls: each MM contracts 256 (two 128-deep k-blocks) per
pass, halving PE time for those phases. Scales (all powers of 2, exact):
weights x32 into e4m3's normal range, exp biased by -ln(64) so
exp(max score 9.1) stays under e4m3's 240 cap; the combined 1024x output
scale is folded into the residual, and LayerNorm's scale-invariance
erases it. Phase order V -> K -> Q+attn (pipelined) -> O with the fp8 Wo
fully SBUF-resident and per-row LN+store chasing the O-proj matmuls.
"""
import numpy as np

B, S, T, H, NH = 4, 2048, 256, 2048, 16
HD = H // NH  # 128
P = 128
MH = S // 2  # rows per core = 1024
LN_EPS = 1e-5
WS = 32.0                      # weight scale into fp8e4
RS = WS * WS                   # residual / output scale (1024)
ISQ = 1.0 / np.sqrt(HD)
EXPB = float(-np.log(64.0))    # exp bias: keeps exp() under e4m3 max 240

_CACHE = {}


def _build2(reps=1, with_mask=True, affine=True):
    from contextlib import ExitStack
    import concourse.bass as bass
    from concourse import bacc
    import concourse.mybir as mybir
    import concourse.tile as tile

    f32 = mybir.dt.float32
    bf16 = mybir.dt.bfloat16
    fp8 = mybir.dt.float8e4
    Alu = mybir.AluOpType
    Act = mybir.ActivationFunctionType
    DR = mybir.MatmulPerfMode.DoubleRow

    nc = bacc.Bacc("TRN2", target_bir_lowering=False, debug=False, num_devices=8)
    KO = H // P  # 16
    XTp = nc.declare_dram_parameter("XT", [H, MH], bf16, isOutput=False)
    Xresp = nc.declare_dram_parameter("Xres", [MH, H], bf16, isOutput=False)
    ATp = nc.declare_dram_parameter("AT", [H, T], bf16, isOutput=False)
    AT8p = nc.declare_dram_parameter("AT8", [H, T], fp8, isOutput=False)
    WqTp = nc.declare_dram_parameter("WqT", [KO, P, KO, P], bf16, isOutput=False)
    WkTp = nc.declare_dram_parameter("WkT", [KO, P, KO, P], bf16, isOutput=False)
    Wv8p = nc.declare_dram_parameter("WvT8", [H, H], fp8, isOutput=False)
    Wo8p = nc.declare_dram_parameter("WoT8", [H, H], fp8, isOutput=False)
    bqp = nc.declare_dram_parameter("bq", [H], f32, isOutput=False)
    bkp = nc.declare_dram_parameter("bk", [H], f32, isOutput=False)
    bvp = nc.declare_dram_parameter("bv", [H], f32, isOutput=False)
    mrowp = nc.declare_dram_parameter("mrow", [1, T], f32, isOutput=False)
    lngp = nc.declare_dram_parameter("ln_g", [H], f32, isOutput=False)
    lnbp = nc.declare_dram_parameter("ln_b", [H], f32, isOutput=False)
    OUTp = nc.declare_dram_parameter("OUT", [MH, H], f32, isOutput=True)

    ctx = ExitStack()
    with tile.TileContext(nc) as tc, ctx:
        if reps > 1:
            ctx.enter_context(tc.For_i(0, reps, 1))
        persist = ctx.enter_context(tc.tile_pool(name="persist", bufs=1))
        wstream = ctx.enter_context(tc.tile_pool(name="wstream", bufs=3))
        wvpool = ctx.enter_context(tc.tile_pool(name="wvpool", bufs=2))
        qpool = ctx.enter_context(tc.tile_pool(name="qpool", bufs=3))
        small = ctx.enter_context(tc.tile_pool(name="small", bufs=1))
        attnsb = ctx.enter_context(tc.tile_pool(name="attnsb", bufs=3))
        sums_p = ctx.enter_context(tc.tile_pool(name="sums", bufs=4))
        bcast = ctx.enter_context(tc.tile_pool(name="bcast", bufs=2))
        orow_p = ctx.enter_context(tc.tile_pool(name="orow", bufs=2))
        xr_p = ctx.enter_context(tc.tile_pool(name="xr", bufs=2))
        psA = ctx.enter_context(tc.tile_pool(name="psA", bufs=4, space="PSUM"))
        psB = ctx.enter_context(tc.tile_pool(name="psB", bufs=4, space="PSUM"))

        # --- constants ---
        eps_t = small.tile([P, 1], f32, tag="eps")
        nc.vector.memset(eps_t[:], LN_EPS)
        ones2 = small.tile([P, 2, P], fp8, tag="ones2")
        nc.vector.memset(ones2[:], 1.0)
        if with_mask:
            # per key position t: mask*-1e9 + EXPB, laid out [t%128, t//128]
            mrow_t = small.tile([P, 2], f32, tag="mrowT")
            nc.sync.dma_start(mrow_t[:], mrowp.ap().rearrange("1 (c p) -> p c", p=P))
        else:
            ebias = small.tile([P, 1], f32, tag="ebias")
            nc.vector.memset(ebias[:], EXPB)
        if affine:
            bq_t = small.tile([P, KO], f32, tag="bq")
            bk_t = small.tile([P, KO], f32, tag="bk")
            nc.sync.dma_start(bq_t[:], bqp.ap().rearrange("(o p) -> p o", p=P))
            nc.sync.dma_start(bk_t[:], bkp.ap().rearrange("(o p) -> p o", p=P))

        def bc_tile(src_ap):
            t = bcast.tile([P, H], f32, tag="bc", name="bc")
            bcast_ap = bass.AP(tensor=src_ap.tensor, offset=src_ap.offset,
                               ap=[[0, P]] + src_ap.ap)
            nc.sync.dma_start(t[:], bcast_ap)
            return t

        bvbc = bc_tile(bvp.ap()) if affine else None

        # --- persistent tensors ---
        at8_t = persist.tile([P, KO, T], fp8, tag="at8", name="at8_t")
        nc.scalar.dma_start(at8_t[:], AT8p.ap().rearrange("(o p) t -> p o t", p=P))
        at_t = persist.tile([P, KO, T], bf16, tag="at", name="at_t")
        kT = persist.tile([P, KO, T], bf16, tag="kT")
        v2 = persist.tile([P, 2, H], fp8, tag="v2", name="v2")
        xt_t = persist.tile([P, KO, MH], bf16, tag="xt", name="xt_t")
        wo8 = persist.tile([P, KO, H], fp8, tag="wo8", name="wo8")
        ctxT = [persist.tile([P, KO, 512], fp8, tag=f"ctx{m}", name=f"ctx{m}")
                for m in range(2)]

        # head DMAs: gpsimd carries xt + wo8 + wq stream; scalar carries
        # at8/at/wv8; sync carries wk stream.
        for q4 in range(4):
            nc.gpsimd.dma_start(
                xt_t[:, q4 * 4:(q4 + 1) * 4],
                XTp.ap()[q4 * 512:(q4 + 1) * 512]
                .rearrange("(o p) m -> p o m", p=P))
        nc.scalar.dma_start(at_t[:], ATp.ap().rearrange("(o p) t -> p o t", p=P))
        for q4 in range(4):
            nc.gpsimd.dma_start(
                wo8[:, q4 * 4:(q4 + 1) * 4],
                Wo8p.ap()[q4 * 512:(q4 + 1) * 512]
                .rearrange("(o p) c -> p o c", p=P))

        # ---------------- V projection (fp8 DoubleRow) ----------------
        def emit_v(n):
            wv = wvpool.tile([P, KO, 512], fp8, tag="wv", name="wv")
            nc.scalar.dma_start(wv[:], Wv8p.ap()[:, n * 512:(n + 1) * 512]
                                .rearrange("(o p) c -> p o c", p=P))
            pool = psA if n % 2 == 0 else psB
            vps = [pool.tile([P, 512], f32, tag="ps", name="vps") for _ in range(2)]
            for t in range(2):
                for j in range(KO // 2):
                    nc.tensor.matmul(vps[t][:],
                                     at8_t[:, 2 * j:2 * j + 2, t * P:(t + 1) * P],
                                     wv[:, 2 * j:2 * j + 2, :],
                                     start=(j == 0), stop=(j == KO // 2 - 1),
                                     perf_mode=DR)
            for t in range(2):
                if affine:
                    nc.vector.tensor_tensor(out=v2[:, t, n * 512:(n + 1) * 512],
                                            in0=vps[t][:],
                                            in1=bvbc[:, n * 512:(n + 1) * 512],
                                            op=Alu.add)
                else:
                    nc.vector.tensor_copy(v2[:, t, n * 512:(n + 1) * 512], vps[t][:])

        # ---------------- K projection (bf16) ----------------
        def emit_k(hp):
            wk = wstream.tile([P, KO, P], bf16, tag="w", name="wk")
            nc.sync.dma_start(wk[:], WkTp.ap()[hp])
            pool = psA if hp % 2 == 0 else psB
            kps = pool.tile([P, 512], f32, tag="ps", name="kps")
            for h in range(KO):
                nc.tensor.matmul(kps[:, :T], wk[:, h], at_t[:, h],
                                 start=(h == 0), stop=(h == KO - 1))
            if affine:
                nc.vector.tensor_tensor(out=kT[:, hp], in0=kps[:, :T],
                                        in1=bk_t[:, hp:hp + 1].to_broadcast((P, T)),
                                        op=Alu.add)
            else:
                nc.vector.tensor_copy(kT[:, hp], kps[:, :T])

        # ---------------- Q projection (bf16) ----------------
        def emit_q_both(hp):
            wq = wstream.tile([P, KO, P], bf16, tag="w", name="wq")
            nc.gpsimd.dma_start(wq[:], WqTp.ap()[hp])
            qp = [psA.tile([P, 512], f32, tag="ps", name="qps"),
                  psB.tile([P, 512], f32, tag="ps", name="qps")]
            for h in range(KO):
                for m in range(2):
                    nc.tensor.matmul(qp[m][:], wq[:, h], xt_t[:, h, m * 512:(m + 1) * 512],
                                     start=(h == 0), stop=(h == KO - 1))
            qt = []
            for m in range(2):
                q_sb = qpool.tile([P, 512], bf16, tag=f"q{m}", name=f"q{m}")
                if affine:
                    nc.vector.tensor_tensor(out=q_sb[:], in0=qp[m][:],
                                            in1=bq_t[:, hp:hp + 1].to_broadcast((P, 512)),
                                            op=Alu.add)
                else:
                    nc.vector.tensor_copy(q_sb[:], qp[m][:])
                qt.append(q_sb)
            return qt

        # attention unit (head hd, query half m): scoresT -> exp(fp8) -> sums/ctx
        def attn_A(hd, m, qh):
            sp = [psA.tile([P, 512], f32, tag="ps", name="sps") for _ in range(2)]
            expT = attnsb.tile([P, 2, 512], fp8, tag="expT", name="expT")
            for c in range(2):
                nc.tensor.matmul(sp[c][:], kT[:, hd, c * P:(c + 1) * P], qh[:],
                                 start=True, stop=True)
                bias_ap = mrow_t[:, c:c + 1] if with_mask else ebias[:]
                nc.scalar.activation(expT[:, c], sp[c][:], Act.Exp,
                                     scale=float(ISQ / RS), bias=bias_ap)
            return (hd, m, expT)

        def attn_B(state):
            hd, m, expT = state
            sm = psA.tile([P, 512], f32, tag="ps", name="smps")
            nc.tensor.matmul(sm[:], ones2[:, 0:2, :], expT[:, 0:2, :],
                             start=True, stop=True, perf_mode=DR)
            cx = psB.tile([P, 512], f32, tag="ps", name="cxps")
            nc.tensor.matmul(cx[:], v2[:, 0:2, hd * P:(hd + 1) * P], expT[:, 0:2, :],
                             start=True, stop=True, perf_mode=DR)
            rb = attnsb.tile([P, 512], f32, tag="rb", name="rb")
            nc.vector.reciprocal(rb[:], sm[:])
            nc.vector.tensor_tensor(out=ctxT[m][:, hd], in0=cx[:], in1=rb[:],
                                    op=Alu.mult)

        # ---------------- O projection (fp8 DoubleRow) + LN ----------------
        gbc = bc_tile(lngp.ap()) if affine else None
        bbc = bc_tile(lnbp.ap()) if affine else None

        def emit_o_ln(mi):
            mh, mc = divmod(mi, 4)
            xr = xr_p.tile([P, H], bf16, tag="xr", name="xr")
            nc.scalar.dma_start(xr[:], Xresp.ap()[mi * P:(mi + 1) * P, :])
            pool = psA if mi % 2 == 0 else psB
            ops = [pool.tile([P, 512], f32, tag="ps", name="ops") for _ in range(4)]
            for j in range(KO // 2):
                for n in range(4):
                    nc.tensor.matmul(ops[n][:],
                                     ctxT[mh][:, 2 * j:2 * j + 2, mc * P:(mc + 1) * P],
                                     wo8[:, 2 * j:2 * j + 2, n * 512:(n + 1) * 512],
                                     start=(j == 0), stop=(j == KO // 2 - 1),
                                     perf_mode=DR)
            row = orow_p.tile([P, H], f32, tag="orow", name="orow")
            stats = sums_p.tile([P, 4, 6], f32, tag="bnst", name="stats")
            for n in range(4):
                nc.vector.tensor_tensor(out=row[:, n * 512:(n + 1) * 512],
                                        in0=ops[n][:],
                                        in1=xr[:, n * 512:(n + 1) * 512], op=Alu.add)
                nc.vector.bn_stats(out=stats[:, n], in_=row[:, n * 512:(n + 1) * 512])
            mv = sums_p.tile([P, 2], f32, tag="bnmv", name="mv")
            nc.vector.bn_aggr(out=mv[:], in_=stats[:])
            std = sums_p.tile([P, 1], f32, tag="std", name="std")
            nc.scalar.activation(std[:], mv[:, 1:2], Act.Sqrt, bias=eps_t[:])
            rstd = sums_p.tile([P, 1], f32, tag="rstd", name="rstd")
            nc.vector.reciprocal(rstd[:], std[:])
            nc.vector.tensor_scalar(out=row[:], in0=row[:], scalar1=mv[:, 0:1],
                                    scalar2=rstd[:], op0=Alu.subtract, op1=Alu.mult)
            if affine:
                nc.vector.tensor_tensor(out=row[:], in0=row[:], in1=gbc[:], op=Alu.mult)
                nc.vector.tensor_tensor(out=row[:], in0=row[:], in1=bbc[:], op=Alu.add)
            oeng = nc.sync if mi % 2 == 0 else nc.gpsimd
            oeng.dma_start(OUTp.ap()[mi * P:(mi + 1) * P, :], row[:])

        # ---------------- schedule ----------------
        for n in range(4):
            emit_v(n)
        for hp in range(KO):
            emit_k(hp)
        # Q proj pipelined with attention one iteration behind
        pend = None
        prev = None
        for hp in range(KO):
            qt = emit_q_both(hp)
            if prev is not None:
                for m in range(2):
                    st = attn_A(prev[0], m, prev[1][m])
                    if pend is not None:
                        attn_B(pend)
                    pend = st
            prev = (hp, qt)
        for m in range(2):
            st = attn_A(prev[0], m, prev[1][m])
            if pend is not None:
                attn_B(pend)
            pend = st
        attn_B(pend)
        for mi in range(8):
            emit_o_ln(mi)

    nc.finalize()
    return nc


def _get_nc(reps=1, with_mask=False, affine=True):
    key = f"nc{reps}_{with_mask}_{affine}"
    if key not in _CACHE:
        _CACHE[key] = _build2(reps, with_mask, affine)
    return _CACHE[key]


_SHARDED = {"XT", "Xres", "AT", "AT8", "mrow"}


def _get_runner(reps=1, with_mask=False, affine=True):
    key = f"runner{reps}_{with_mask}_{affine}"
    if key in _CACHE:
        return _CACHE[key]
    import jax
    from jax.sharding import Mesh, PartitionSpec, NamedSharding
    try:
        from jax.experimental.shard_map import shard_map
    except ImportError:
        from jax import shard_map
    from concourse.bass2jax import (_bass_exec_p, partition_id_tensor,
                                    install_neuronx_cc_hook)
    import concourse.mybir as mybir

    install_neuronx_cc_hook()
    nc = _get_nc(reps, with_mask, affine)
    partition_name = nc.partition_id_tensor.name if nc.partition_id_tensor else None
    in_names, out_names, out_avals = [], [], []
    for alloc in nc.m.functions[0].allocations:
        if not isinstance(alloc, mybir.MemoryLocationSet):
            continue
        name = alloc.memorylocations[0].name
        if alloc.kind == "ExternalInput":
            if name != partition_name:
                in_names.append(name)
        elif alloc.kind == "ExternalOutput":
            out_names.append(name)
            out_avals.append(jax.core.ShapedArray(tuple(alloc.tensor_shape),
                                                  mybir.dt.np(alloc.dtype)))

    bind_in_names = list(in_names) + ([partition_name] if partition_name else [])

    def _body(*args):
        operands = list(args)
        if partition_name is not None:
            operands.append(partition_id_tensor())
        outs = _bass_exec_p.bind(
            *operands, out_avals=tuple(out_avals),
            in_names=tuple(bind_in_names), out_names=tuple(out_names),
            lowering_input_output_aliases=(),
            sim_require_finite=True, sim_require_nnan=True, nc=nc)
        return tuple(outs)

    devices = jax.devices()[:8]
    mesh = Mesh(np.asarray(devices), ("core",))
    in_specs = tuple(PartitionSpec("core") if n in _SHARDED else PartitionSpec()
                     for n in in_names)
    out_specs = tuple(PartitionSpec("core") for _ in out_names)
    fn = jax.jit(shard_map(_body, mesh=mesh, in_specs=in_specs,
                           out_specs=out_specs, check_rep=False),
                 keep_unused=True)
    shardings = {n: NamedSharding(mesh, s) for n, s in zip(in_names, in_specs)}
    _CACHE[key] = (fn, in_names, mesh, shardings)
    return _CACHE[key]


def _host_args(hidden_states, audio_tokens, attention_mask, Wq, bq, Wk, bk, Wv,
               bv, Wo, bo, ln_g, ln_b):
    import ml_dtypes
    bf = ml_dtypes.bfloat16
    e4 = ml_dtypes.float8_e4m3
    hs = np.asarray(hidden_states, np.float32)
    at = np.asarray(audio_tokens, np.float32)
    am = np.asarray(attention_mask, np.float32)
    Wq = np.asarray(Wq, np.float32); Wk = np.asarray(Wk, np.float32)
    Wv = np.asarray(Wv, np.float32); Wo = np.asarray(Wo, np.float32)
    bq = np.asarray(bq, np.float32); bk = np.asarray(bk, np.float32)
    bv = np.asarray(bv, np.float32); bo = np.asarray(bo, np.float32)
    ln_g = np.asarray(ln_g, np.float32); ln_b = np.asarray(ln_b, np.float32)

    KO_ = H // P

    def _tile_w(WT):
        # [h, h'] -> [hp, p, o, c] with h = o*128+p, h' = hp*128+c
        return np.ascontiguousarray(
            WT.reshape(KO_, P, KO_, P).transpose(2, 1, 0, 3)).astype(bf)

    vals = {
        "WqT": _tile_w(Wq.T * WS), "WkT": _tile_w(Wk.T * WS),
        "WvT8": np.ascontiguousarray(Wv.T * WS).astype(e4),
        "WoT8": np.ascontiguousarray(Wo.T * WS).astype(e4),
        "bq": bq * WS, "bk": bk * WS, "bv": bv * WS,
        "ln_g": ln_g, "ln_b": ln_b,
    }
    xts, xrs, ats, at8s, mrs = [], [], [], [], []
    for c in range(8):
        b, half = divmod(c, 2)
        xs = hs[b, half * MH:(half + 1) * MH]
        xts.append(xs.T.astype(bf))
        xrs.append(((xs + bo) * RS).astype(bf))
        ats.append(at[b].T.astype(bf))
        at8s.append(at[b].T.astype(e4))
        mrs.append((am[b] * -1e9 + EXPB).reshape(1, T).astype(np.float32))
    vals["XT"] = np.concatenate(xts, axis=0)
    vals["Xres"] = np.concatenate(xrs, axis=0)
    vals["AT"] = np.concatenate(ats, axis=0)
    vals["AT8"] = np.concatenate(at8s, axis=0)
    vals["mrow"] = np.concatenate(mrs, axis=0)
    return vals


def _assemble(out_global):
    o = np.asarray(out_global).reshape(8, MH, H)
    out = np.empty((B, S, H), np.float32)
    for c in range(8):
        b, half = divmod(c, 2)
        out[b, half * MH:(half + 1) * MH] = o[c]
    return out


def _flags(inputs):
    with_mask = bool(np.any(np.asarray(inputs["attention_mask"]) != 0))
    affine = not (np.all(np.asarray(inputs["bq"]) == 0)
                  and np.all(np.asarray(inputs["bk"]) == 0)
                  and np.all(np.asarray(inputs["bv"]) == 0)
                  and np.all(np.asarray(inputs["ln_g"]) == 1)
                  and np.all(np.asarray(inputs["ln_b"]) == 0))
    return with_mask, affine


def kernel(**inputs):
    with_mask, affine = _flags(inputs)
    fn, in_names, mesh, shardings = _get_runner(1, with_mask, affine)
    vals = _host_args(**inputs)
    outs = fn(*[vals[n] for n in in_names])
    return _assemble(outs[0])


def device_args(inputs, reps=1):
    """device_put all inputs once; returns list for run_device."""
    import jax
    with_mask, affine = _flags(inputs)
    fn, in_names, mesh, shardings = _get_runner(reps, with_mask, affine)
    vals = _host_args(**inputs)
    return [jax.device_put(vals[n], shardings[n]) for n in in_names]


def run_device(args, reps=1, with_mask=False, affine=False):
    import jax
    fn, in_names, mesh, shardings = _get_runner(reps, with_mask, affine)
    outs = fn(*args)
    jax.block_until_ready(outs)
    return outs
